# revision 6
# baseline (speedup 1.0000x reference)
"""Householder reflection kernel for Trainium2, data-parallel over 8 NeuronCores.

out = z - 2 * v * (v.z) / (v.v), rowwise over [8192, 2048] f32.

Sharding: batch dim split 8 ways (1024 rows/core); no cross-core communication.
Per core, each 128-row tile needs two rowwise reductions (v.z and v.v) and one
fused multiply-add. The kernel is HBM-bandwidth bound (24 MiB of traffic per
core); compute is spread over DVE + ACT so it hides under DMA.
"""

from contextlib import ExitStack

import numpy as np

import concourse.bacc as bacc
import concourse.bass as bass
import concourse.tile as tile
from concourse import mybir
from concourse.bass_utils import run_bass_kernel_spmd

N_CORES = 8
B, L = 8192, 2048
RPC = B // N_CORES  # rows per core
P = 128             # SBUF partitions
N_TILES = RPC // P  # row tiles per core

_NC = None
VARIANT = "stt_accum"


def build_nc(variant: str = VARIANT) -> bass.Bass:
    nc = bacc.Bacc("TRN2")
    f32 = mybir.dt.float32
    v = nc.declare_dram_parameter("v", [RPC, L], f32, isOutput=False)
    z = nc.declare_dram_parameter("z", [RPC, L], f32, isOutput=False)
    out = nc.declare_dram_parameter("out", [RPC, L], f32, isOutput=True)

    with tile.TileContext(nc) as tc, ExitStack() as ctx:
        vpool = ctx.enter_context(tc.tile_pool(name="vp", bufs=3))
        zpool = ctx.enter_context(tc.tile_pool(name="zp", bufs=3))
        opool = ctx.enter_context(tc.tile_pool(name="op", bufs=3))
        spool = ctx.enter_context(tc.tile_pool(name="sp", bufs=2))
        stats = ctx.enter_context(tc.tile_pool(name="st", bufs=4))

        for i in range(N_TILES):
            rows = slice(i * P, (i + 1) * P)
            vt = vpool.tile([P, L], f32)
            nc.sync.dma_start(vt[:], v[rows])
            zt = zpool.tile([P, L], f32)
            nc.sync.dma_start(zt[:], z[rows])

            vz = stats.tile([P, 1], f32, tag="vz")
            if variant == "stt_accum":
                # one DVE pass: prod = (v bypass 1.0) * z ; vz = rowsum(prod)
                prod = spool.tile([P, L], f32, tag="prod")
                nc.vector.scalar_tensor_tensor(
                    out=prod[:], in0=vt[:], scalar=1.0, in1=zt[:],
                    op0=mybir.AluOpType.bypass, op1=mybir.AluOpType.mult,
                    accum_out=vz[:],
                )
            else:
                prod = spool.tile([P, L], f32, tag="prod")
                nc.vector.tensor_mul(prod[:], vt[:], zt[:])
                nc.vector.tensor_reduce(
                    vz[:], prod[:], mybir.AxisListType.X, mybir.AluOpType.add
                )

            sq = spool.tile([P, L], f32, tag="sq")
            nsq = stats.tile([P, 1], f32, tag="nsq")
            nc.scalar.activation(
                out=sq[:], in_=vt[:],
                func=mybir.ActivationFunctionType.Square,
                accum_out=nsq[:],
            )

            r = stats.tile([P, 1], f32, tag="r")
            nc.vector.reciprocal(r[:], nsq[:])
            s = stats.tile([P, 1], f32, tag="s")
            nc.vector.scalar_tensor_tensor(
                out=s[:], in0=vz[:], scalar=-2.0, in1=r[:],
                op0=mybir.AluOpType.mult, op1=mybir.AluOpType.mult,
            )

            ot = opool.tile([P, L], f32)
            nc.vector.scalar_tensor_tensor(
                out=ot[:], in0=vt[:], scalar=s[:], in1=zt[:],
                op0=mybir.AluOpType.mult, op1=mybir.AluOpType.add,
            )
            nc.sync.dma_start(out[rows], ot[:])

    nc.compile()  # bacc: split sync waits, alloc regs, fuse nops
    return nc


def _get_nc() -> bass.Bass:
    global _NC
    if _NC is None:
        _NC = build_nc()
    return _NC


def _in_maps(v: np.ndarray, z: np.ndarray) -> list[dict]:
    v = np.ascontiguousarray(np.asarray(v, dtype=np.float32))
    z = np.ascontiguousarray(np.asarray(z, dtype=np.float32))
    return [
        {"v": v[i * RPC : (i + 1) * RPC], "z": z[i * RPC : (i + 1) * RPC]}
        for i in range(N_CORES)
    ]


def run_spmd(v: np.ndarray, z: np.ndarray, **kwargs):
    """Run on all 8 cores; returns BassKernelResults (kwargs e.g. trace=True)."""
    return run_bass_kernel_spmd(_get_nc(), _in_maps(v, z), list(range(N_CORES)), **kwargs)


def kernel(v: np.ndarray, z: np.ndarray) -> np.ndarray:
    res = run_spmd(v, z)
    return np.concatenate([res.results[i]["out"] for i in range(N_CORES)], axis=0)


# revision 7
# speedup vs baseline: 1.0989x; 1.0989x over previous
"""Householder reflection kernel for Trainium2, data-parallel over 8 NeuronCores.

out = z - 2 * v * (v.z) / (v.v), rowwise over [8192, 2048] f32.

Sharding: batch dim split 8 ways (1024 rows/core); no cross-core communication.
HBM-bandwidth bound (24 MiB of traffic per core). Structure per core:
  - macro tiles of 256 rows: one 2 MiB DMA per tensor ([128, 2, 2048] SBUF
    tile, partition p holding rows p and p+128) for high DMA efficiency
  - loads issued from the Sync (SP) HWDGE queue, stores from the GpSimd
    (SWDGE) queue so store waits never head-of-line block later loads
  - per 128-row block: DVE scalar_tensor_tensor computes v*z with rowsum
    accum (vz), ACT activation(Square) computes rowsum(v^2) (nsq),
    DVE reciprocal + tiny STT give s = -2*vz/nsq, one fused DVE STT
    computes out = v*s + z.
"""

from contextlib import ExitStack

import numpy as np

import concourse.bacc as bacc
import concourse.bass as bass
import concourse.tile as tile
from concourse import mybir
from concourse.bass_utils import run_bass_kernel_spmd

N_CORES = 8
B, L = 8192, 2048
RPC = B // N_CORES   # rows per core
P = 128              # SBUF partitions
TPB = 2              # row-blocks per macro tile (DMA size = TPB MiB)
N_MACRO = RPC // (P * TPB)

_NC = None


def build_nc() -> bass.Bass:
    nc = bacc.Bacc("TRN2")
    f32 = mybir.dt.float32
    v = nc.declare_dram_parameter("v", [RPC, L], f32, isOutput=False)
    z = nc.declare_dram_parameter("z", [RPC, L], f32, isOutput=False)
    out = nc.declare_dram_parameter("out", [RPC, L], f32, isOutput=True)

    with tile.TileContext(nc) as tc, ExitStack() as ctx:
        vpool = ctx.enter_context(tc.tile_pool(name="vp", bufs=3))
        zpool = ctx.enter_context(tc.tile_pool(name="zp", bufs=3))
        opool = ctx.enter_context(tc.tile_pool(name="op", bufs=3))
        spool = ctx.enter_context(tc.tile_pool(name="sp", bufs=1))
        stats = ctx.enter_context(tc.tile_pool(name="st", bufs=2 * TPB))

        # write-only sinks for the reduction ops' full outputs (never read)
        prod_sink = spool.tile([P, L], f32, tag="prod")
        sq_sink = spool.tile([P, L], f32, tag="sq")

        rows_per_macro = P * TPB
        for i in range(N_MACRO):
            r0 = i * rows_per_macro
            src_v = v[r0 : r0 + rows_per_macro].rearrange("(a p) m -> p a m", p=P)
            src_z = z[r0 : r0 + rows_per_macro].rearrange("(a p) m -> p a m", p=P)
            dst_o = out[r0 : r0 + rows_per_macro].rearrange("(a p) m -> p a m", p=P)

            vt = vpool.tile([P, TPB, L], f32)
            nc.sync.dma_start(vt[:], src_v)
            zt = zpool.tile([P, TPB, L], f32)
            nc.sync.dma_start(zt[:], src_z)
            ot = opool.tile([P, TPB, L], f32)

            for a in range(TPB):
                va = vt[:, a, :]
                za = zt[:, a, :]

                vz = stats.tile([P, 1], f32, tag="vz")
                nc.vector.scalar_tensor_tensor(
                    out=prod_sink[:], in0=va, scalar=1.0, in1=za,
                    op0=mybir.AluOpType.bypass, op1=mybir.AluOpType.mult,
                    accum_out=vz[:],
                )

                nsq = stats.tile([P, 1], f32, tag="nsq")
                nc.scalar.activation(
                    out=sq_sink[:], in_=va,
                    func=mybir.ActivationFunctionType.Square,
                    accum_out=nsq[:],
                )

                r = stats.tile([P, 1], f32, tag="r")
                nc.vector.reciprocal(r[:], nsq[:])
                s = stats.tile([P, 1], f32, tag="s")
                nc.vector.scalar_tensor_tensor(
                    out=s[:], in0=vz[:], scalar=-2.0, in1=r[:],
                    op0=mybir.AluOpType.mult, op1=mybir.AluOpType.mult,
                )

                nc.vector.scalar_tensor_tensor(
                    out=ot[:, a, :], in0=va, scalar=s[:], in1=za,
                    op0=mybir.AluOpType.mult, op1=mybir.AluOpType.add,
                )

            nc.gpsimd.dma_start(dst_o, ot[:])

    nc.compile()  # bacc: split sync waits, alloc regs, fuse nops
    return nc


def _get_nc() -> bass.Bass:
    global _NC
    if _NC is None:
        _NC = build_nc()
    return _NC


def _in_maps(v: np.ndarray, z: np.ndarray) -> list[dict]:
    v = np.ascontiguousarray(np.asarray(v, dtype=np.float32))
    z = np.ascontiguousarray(np.asarray(z, dtype=np.float32))
    return [
        {"v": v[i * RPC : (i + 1) * RPC], "z": z[i * RPC : (i + 1) * RPC]}
        for i in range(N_CORES)
    ]


def run_spmd(v: np.ndarray, z: np.ndarray, **kwargs):
    """Run on all 8 cores; returns BassKernelResults (kwargs e.g. trace=True)."""
    return run_bass_kernel_spmd(_get_nc(), _in_maps(v, z), list(range(N_CORES)), **kwargs)


def kernel(v: np.ndarray, z: np.ndarray) -> np.ndarray:
    res = run_spmd(v, z)
    return np.concatenate([res.results[i]["out"] for i in range(N_CORES)], axis=0)
